# revision 2
# baseline (speedup 1.0000x reference)
"""Trainium2 Bass kernel for nn_ChannelRandomPaddingSkip.

Computes out[:, j] = 0.25 * x[:, perm[j]] for x (32, 64, 128, 128) f32,
perm (256,) int32, out (32, 256, 128, 128) f32.

Strategy: pure data-parallel over batch (4 images per core, 8 cores).
Per core:
  - SBUF tile layout T[p, c, e] with partition p = (b, s): b in [0,4) batch,
    s in [0,32) segments of the 16384-elem image; c in [0,64) channels;
    e in [0,512) elems. Every DRAM run is 2KiB contiguous and every DMA
    spans all 128 partitions.
  - Input: 8 chunked DMAs (8 channels / 2MiB each), scaled 0.25 in place
    on the vector engine.
  - Output: 256 gather DMAs (one per output channel, 256KiB each), source
    channel baked in host-side from the perm values.
"""

import sys

for _p in ("/opt/trn_rl_repo", "/root/.axon_site/_ro/trn_rl_repo"):
    if _p not in sys.path:
        sys.path.append(_p)

import numpy as np

B, C_IN, H, W = 32, 64, 128, 128
C_OUT = 256
N_CORES = 8
B_LOC = B // N_CORES          # 4 batches per core
HW = H * W                    # 16384
SEG = 32                      # segments per image -> 4*32 = 128 partitions
E = HW // SEG                 # 512 elems (2KiB) per segment
H2 = H // SEG                 # rows per segment
C_CHUNK = 8                   # input channels per load chunk
SCALE = 0.25

_cache = {}


def _build(perm_key):
    import concourse.bacc as bacc
    import concourse.tile as tile
    from concourse import mybir

    perm = list(perm_key)
    nc = bacc.Bacc("TRN2", target_bir_lowering=False, debug=False)
    x = nc.dram_tensor("x", [B_LOC, C_IN, H, W], mybir.dt.float32,
                       kind="ExternalInput")
    out = nc.dram_tensor("out", [B_LOC, C_OUT, H, W], mybir.dt.float32,
                         kind="ExternalOutput")

    # (b, s, c, e) views; for a fixed channel the (s, e) dims merge into one
    # contiguous 64KiB run per batch, so each per-channel DMA is a 3-dim AP:
    # DRAM (4, 32, 512) <-> SBUF (128, 512).
    x_v = x.ap().rearrange("b c (s h2) w -> b s c (h2 w)", s=SEG, h2=H2)
    out_v = out.ap().rearrange("b j (s h2) w -> b s j (h2 w)", s=SEG, h2=H2)

    # Output channels grouped by source channel, so stores can start as soon
    # as their channel is loaded and scaled.
    by_src = [[] for _ in range(C_IN)]
    for j in range(C_OUT):
        by_src[perm[j]].append(j)

    with tile.TileContext(nc) as tc:
        with tc.tile_pool(name="chan", bufs=1) as pool:
            tiles = []
            for c in range(C_IN):
                t = pool.tile([128, E], mybir.dt.float32,
                              name=f"ch{c}", tag=f"ch{c}")
                tiles.append(t)
            k = 0
            for c in range(C_IN):
                t = tiles[c]
                nc.sync.dma_start(t[:], x_v[:, :, c, :])
                nc.vector.tensor_scalar_mul(t[:], t[:], SCALE)
                for j in by_src[c]:
                    eng = nc.sync if k % 2 == 0 else nc.scalar
                    eng.dma_start(out_v[:, :, j, :], t[:])
                    k += 1
    nc.compile()
    return nc


def _get_nc(perm_key):
    nc = _cache.get(perm_key)
    if nc is None:
        nc = _build(perm_key)
        _cache[perm_key] = nc
    return nc


def kernel(x, perm):
    from concourse.bass_utils import run_bass_kernel_spmd

    x = np.ascontiguousarray(np.asarray(x), dtype=np.float32)
    perm_np = np.asarray(perm)
    nc = _get_nc(tuple(int(v) for v in perm_np.tolist()))

    in_maps = [{"x": x[i * B_LOC:(i + 1) * B_LOC]} for i in range(N_CORES)]
    res = run_bass_kernel_spmd(nc, in_maps, core_ids=list(range(N_CORES)))
    return np.concatenate([res.results[i]["out"] for i in range(N_CORES)],
                          axis=0)


# revision 3
# speedup vs baseline: 19731.3818x; 19731.3818x over previous
"""Trainium2 Bass kernel for nn_ChannelRandomPaddingSkip.

Computes out[:, j] = 0.25 * x[:, perm[j]] for x (32, 64, 128, 128) f32,
perm (256,) int32, out (32, 256, 128, 128) f32.

Strategy: pure data-parallel over batch (4 images per core, 8 cores), no
cross-core communication. Per core:
  - SBUF layout: one tile per input channel, [128, 512] f32, partition
    p = (b, s): b in [0,4) batch, s in [0,32) segments of the 16384-elem
    image plane. Every DMA spans all 128 partitions with 2KiB contiguous
    runs on both the DRAM and SBUF side.
  - 64 channel loads (256KiB each), scaled by 0.25 in place on the vector
    engine, then 256 gather stores (256KiB each) whose source channel is
    baked in host-side from the perm values. Stores start as soon as their
    source channel is resident; the Tile scheduler overlaps everything.
"""

import sys

for _p in ("/opt/trn_rl_repo", "/root/.axon_site/_ro/trn_rl_repo"):
    if _p not in sys.path:
        sys.path.append(_p)

import numpy as np

B, C_IN, H, W = 32, 64, 128, 128
C_OUT = 256
N_CORES = 8
B_LOC = B // N_CORES          # 4 batches per core
HW = H * W                    # 16384
SEG = 32                      # segments per image -> 4*32 = 128 partitions
E = HW // SEG                 # 512 elems (2KiB) per segment
H2 = H // SEG                 # rows per segment
SCALE = 0.25

_cache = {}


def _emit_body(nc, tc, tile_mod, mybir, pool, x_v, out_v, perm, by_src):
    tiles = []
    for c in range(C_IN):
        t = pool.tile([128, E], mybir.dt.float32,
                      name=f"ch{c}", tag=f"ch{c}")
        tiles.append(t)
    k = 0
    for c in range(C_IN):
        t = tiles[c]
        nc.sync.dma_start(t[:], x_v[:, :, c, :])
        nc.vector.tensor_scalar_mul(t[:], t[:], SCALE)
        for j in by_src[c]:
            eng = nc.sync if k % 2 == 0 else nc.scalar
            eng.dma_start(out_v[:, :, j, :], t[:])
            k += 1


def build(perm_key, reps=1):
    """Build + compile the per-core program. reps>1 wraps the body in an
    on-device loop (used only by the timing harness)."""
    import concourse.bacc as bacc
    import concourse.tile as tile
    from concourse import mybir

    perm = list(perm_key)
    nc = bacc.Bacc("TRN2", target_bir_lowering=False, debug=False)
    x = nc.dram_tensor("x", [B_LOC, C_IN, H, W], mybir.dt.float32,
                       kind="ExternalInput")
    out = nc.dram_tensor("out", [B_LOC, C_OUT, H, W], mybir.dt.float32,
                         kind="ExternalOutput")

    # (b, s, c, e) views; for a fixed channel the (s, e) dims merge into one
    # contiguous 64KiB run per batch, so each per-channel DMA is a 3-dim AP:
    # DRAM (4, 32, 512) <-> SBUF (128, 512).
    x_v = x.ap().rearrange("b c (s h2) w -> b s c (h2 w)", s=SEG, h2=H2)
    out_v = out.ap().rearrange("b j (s h2) w -> b s j (h2 w)", s=SEG, h2=H2)

    # Output channels grouped by source channel, so stores can start as soon
    # as their channel is loaded and scaled.
    by_src = [[] for _ in range(C_IN)]
    for j in range(C_OUT):
        by_src[perm[j]].append(j)

    with tile.TileContext(nc) as tc:
        with tc.tile_pool(name="chan", bufs=1) as pool:
            if reps == 1:
                _emit_body(nc, tc, tile, mybir, pool, x_v, out_v, perm, by_src)
            else:
                with tc.For_i(0, reps, 1):
                    _emit_body(nc, tc, tile, mybir, pool, x_v, out_v, perm,
                               by_src)
    nc.compile()
    return nc


def _make_runner(nc):
    """Build the sharded jit callable once (via the library's own path, so
    the custom-call lowering matches exactly), and return a fast runner."""
    import jax
    from concourse import bass2jax

    captured = []
    orig_jit = bass2jax.jax.jit

    def spy_jit(*a, **k):
        f = orig_jit(*a, **k)
        captured.append(f)
        return f

    dummy = np.zeros((B_LOC, C_IN, H, W), np.float32)
    bass2jax.jax.jit = spy_jit
    try:
        bass2jax.run_bass_via_pjrt(
            nc, [{"x": dummy} for _ in range(N_CORES)], n_cores=N_CORES)
    finally:
        bass2jax.jax.jit = orig_jit
    sharded = captured[-1]

    out_shape = (N_CORES * B_LOC, C_OUT, H, W)

    def run(x_full):
        zout = np.zeros(out_shape, np.float32)
        r = sharded(x_full, zout)
        return np.asarray(r[0])

    run.sharded = sharded
    return run


def _get_entry(perm_key):
    entry = _cache.get(perm_key)
    if entry is None:
        nc = build(perm_key)
        entry = {"nc": nc, "run": _make_runner(nc)}
        _cache[perm_key] = entry
    return entry


def kernel(x, perm):
    x = np.ascontiguousarray(np.asarray(x), dtype=np.float32)
    perm_np = np.asarray(perm)
    entry = _get_entry(tuple(int(v) for v in perm_np.tolist()))
    return entry["run"](x)


# revision 4
# speedup vs baseline: 50056.7534x; 2.5369x over previous
"""Trainium2 Bass kernel for nn_ChannelRandomPaddingSkip.

Computes out[:, j] = 0.25 * x[:, perm[j]] for x (32, 64, 128, 128) f32,
perm (256,) int32, out (32, 256, 128, 128) f32.

Strategy: pure data-parallel over batch (4 images per core, 8 cores), no
cross-core communication. Per core:
  - SBUF layout: one tile per input channel, [128, 512] f32, partition
    p = (b, s): b in [0,4) batch, s in [0,32) segments of the 16384-elem
    image plane. Every DMA spans all 128 partitions with 2KiB contiguous
    runs on both the DRAM and SBUF side.
  - 64 channel loads (256KiB each), scaled by 0.25 in place on the vector
    engine, then 256 gather stores (256KiB each) whose source channel is
    baked in host-side from the perm values. Stores start as soon as their
    source channel is resident; the Tile scheduler overlaps everything.
"""

import sys

for _p in ("/opt/trn_rl_repo", "/root/.axon_site/_ro/trn_rl_repo"):
    if _p not in sys.path:
        sys.path.append(_p)

import numpy as np

B, C_IN, H, W = 32, 64, 128, 128
C_OUT = 256
N_CORES = 8
B_LOC = B // N_CORES          # 4 batches per core
HW = H * W                    # 16384
SEG = 32                      # segments per image -> 4*32 = 128 partitions
E = HW // SEG                 # 512 elems (2KiB) per segment
H2 = H // SEG                 # rows per segment
SCALE = 0.25

_cache = {}


def _emit_body(nc, tc, tile_mod, mybir, pool, x_v, out_v, perm, by_src):
    tiles = []
    for c in range(C_IN):
        t = pool.tile([128, E], mybir.dt.float32,
                      name=f"ch{c}", tag=f"ch{c}")
        tiles.append(t)
    k = 0
    for c in range(C_IN):
        t = tiles[c]
        nc.sync.dma_start(t[:], x_v[:, :, c, :])
        nc.vector.tensor_scalar_mul(t[:], t[:], SCALE)
        for j in by_src[c]:
            eng = nc.sync if k % 2 == 0 else nc.scalar
            eng.dma_start(out_v[:, :, j, :], t[:])
            k += 1


def build(perm_key, reps=1):
    """Build + compile the per-core program. reps>1 wraps the body in an
    on-device loop (used only by the timing harness)."""
    import concourse.bacc as bacc
    import concourse.tile as tile
    from concourse import mybir

    perm = list(perm_key)
    nc = bacc.Bacc("TRN2", target_bir_lowering=False, debug=False)
    x = nc.dram_tensor("x", [B_LOC, C_IN, H, W], mybir.dt.float32,
                       kind="ExternalInput")
    out = nc.dram_tensor("out", [B_LOC, C_OUT, H, W], mybir.dt.float32,
                         kind="ExternalOutput")

    # (b, s, c, e) views; for a fixed channel the (s, e) dims merge into one
    # contiguous 64KiB run per batch, so each per-channel DMA is a 3-dim AP:
    # DRAM (4, 32, 512) <-> SBUF (128, 512).
    x_v = x.ap().rearrange("b c (s h2) w -> s b c (h2 w)", s=SEG, h2=H2)
    out_v = out.ap().rearrange("b j (s h2) w -> s b j (h2 w)", s=SEG, h2=H2)

    # Output channels grouped by source channel, so stores can start as soon
    # as their channel is loaded and scaled.
    by_src = [[] for _ in range(C_IN)]
    for j in range(C_OUT):
        by_src[perm[j]].append(j)

    with tile.TileContext(nc) as tc:
        with tc.tile_pool(name="chan", bufs=1) as pool:
            if reps == 1:
                _emit_body(nc, tc, tile, mybir, pool, x_v, out_v, perm, by_src)
            else:
                with tc.For_i(0, reps, 1):
                    _emit_body(nc, tc, tile, mybir, pool, x_v, out_v, perm,
                               by_src)
    nc.compile()
    return nc


def _make_runner(nc):
    """Build the sharded jit callable once (via the library's own path, so
    the custom-call lowering matches exactly), and return a fast runner."""
    import jax
    from concourse import bass2jax

    captured = []
    orig_jit = bass2jax.jax.jit

    def spy_jit(*a, **k):
        f = orig_jit(*a, **k)
        captured.append(f)
        return f

    dummy = np.zeros((B_LOC, C_IN, H, W), np.float32)
    bass2jax.jax.jit = spy_jit
    try:
        bass2jax.run_bass_via_pjrt(
            nc, [{"x": dummy} for _ in range(N_CORES)], n_cores=N_CORES)
    finally:
        bass2jax.jax.jit = orig_jit
    sharded = captured[-1]

    out_shape = (N_CORES * B_LOC, C_OUT, H, W)

    def run(x_full):
        zout = np.zeros(out_shape, np.float32)
        r = sharded(x_full, zout)
        return np.asarray(r[0])

    run.sharded = sharded
    return run


def _get_entry(perm_key):
    entry = _cache.get(perm_key)
    if entry is None:
        nc = build(perm_key)
        entry = {"nc": nc, "run": _make_runner(nc)}
        _cache[perm_key] = entry
    return entry


def kernel(x, perm):
    x = np.ascontiguousarray(np.asarray(x), dtype=np.float32)
    perm_np = np.asarray(perm)
    entry = _get_entry(tuple(int(v) for v in perm_np.tolist()))
    return entry["run"](x)


# revision 5
# speedup vs baseline: 58410.4959x; 1.1669x over previous
"""Trainium2 Bass kernel for nn_ChannelRandomPaddingSkip.

Computes out[:, j] = 0.25 * x[:, perm[j]] for x (32, 64, 128, 128) f32,
perm (256,) int32, out (32, 256, 128, 128) f32.

Strategy: pure data-parallel over batch (4 images per core, 8 cores), no
cross-core communication. Per core:
  - SBUF layout: one tile per input channel, [128, 512] f32, partition
    p = (b, s): b in [0,4) batch, s in [0,32) segments of the 16384-elem
    image plane. Every DMA spans all 128 partitions with 2KiB contiguous
    runs on both the DRAM and SBUF side.
  - 64 channel loads (256KiB each), scaled by 0.25 in place on the vector
    engine, then 256 gather stores (256KiB each) whose source channel is
    baked in host-side from the perm values. Stores start as soon as their
    source channel is resident; the Tile scheduler overlaps everything.
"""

import sys

for _p in ("/opt/trn_rl_repo", "/root/.axon_site/_ro/trn_rl_repo"):
    if _p not in sys.path:
        sys.path.append(_p)

import numpy as np

B, C_IN, H, W = 32, 64, 128, 128
C_OUT = 256
N_CORES = 8
B_LOC = B // N_CORES          # 4 batches per core
HW = H * W                    # 16384
SEG = 32                      # segments per image -> 4*32 = 128 partitions
E = HW // SEG                 # 512 elems (2KiB) per segment
H2 = H // SEG                 # rows per segment
SCALE = 0.25

_cache = {}


def _emit_body(nc, tc, tile_mod, mybir, pool, x_v, out_v, perm, by_src):
    tiles = []
    for c in range(C_IN):
        t = pool.tile([128, E], mybir.dt.float32,
                      name=f"ch{c}", tag=f"ch{c}")
        tiles.append(t)
    for c in range(C_IN):
        t = tiles[c]
        # Loads on SWDGE (gpsimd) keep the HWDGE ring dedicated to stores;
        # measured best with all stores on nc.sync (splitting across the
        # scalar ring was slower).
        nc.gpsimd.dma_start(t[:], x_v[:, :, c, :])
        nc.vector.tensor_scalar_mul(t[:], t[:], SCALE)
        for j in by_src[c]:
            nc.sync.dma_start(out_v[:, :, j, :], t[:])


def build(perm_key, reps=1):
    """Build + compile the per-core program. reps>1 wraps the body in an
    on-device loop (used only by the timing harness)."""
    import concourse.bacc as bacc
    import concourse.tile as tile
    from concourse import mybir

    perm = list(perm_key)
    nc = bacc.Bacc("TRN2", target_bir_lowering=False, debug=False)
    x = nc.dram_tensor("x", [B_LOC, C_IN, H, W], mybir.dt.float32,
                       kind="ExternalInput")
    out = nc.dram_tensor("out", [B_LOC, C_OUT, H, W], mybir.dt.float32,
                         kind="ExternalOutput")

    # (b, s, c, e) views; for a fixed channel the (s, e) dims merge into one
    # contiguous 64KiB run per batch, so each per-channel DMA is a 3-dim AP:
    # DRAM (4, 32, 512) <-> SBUF (128, 512).
    x_v = x.ap().rearrange("b c (s h2) w -> s b c (h2 w)", s=SEG, h2=H2)
    out_v = out.ap().rearrange("b j (s h2) w -> s b j (h2 w)", s=SEG, h2=H2)

    # Output channels grouped by source channel, so stores can start as soon
    # as their channel is loaded and scaled.
    by_src = [[] for _ in range(C_IN)]
    for j in range(C_OUT):
        by_src[perm[j]].append(j)

    with tile.TileContext(nc) as tc:
        with tc.tile_pool(name="chan", bufs=1) as pool:
            if reps == 1:
                _emit_body(nc, tc, tile, mybir, pool, x_v, out_v, perm, by_src)
            else:
                with tc.For_i(0, reps, 1):
                    _emit_body(nc, tc, tile, mybir, pool, x_v, out_v, perm,
                               by_src)
    nc.compile()
    return nc


def _make_runner(nc):
    """Build the sharded jit callable once (via the library's own path, so
    the custom-call lowering matches exactly), and return a fast runner."""
    import jax
    from concourse import bass2jax

    captured = []
    orig_jit = bass2jax.jax.jit

    def spy_jit(*a, **k):
        f = orig_jit(*a, **k)
        captured.append(f)
        return f

    dummy = np.zeros((B_LOC, C_IN, H, W), np.float32)
    bass2jax.jax.jit = spy_jit
    try:
        bass2jax.run_bass_via_pjrt(
            nc, [{"x": dummy} for _ in range(N_CORES)], n_cores=N_CORES)
    finally:
        bass2jax.jax.jit = orig_jit
    sharded = captured[-1]

    out_shape = (N_CORES * B_LOC, C_OUT, H, W)

    def run(x_full):
        zout = np.zeros(out_shape, np.float32)
        r = sharded(x_full, zout)
        return np.asarray(r[0])

    run.sharded = sharded
    return run


def _get_entry(perm_key):
    entry = _cache.get(perm_key)
    if entry is None:
        nc = build(perm_key)
        entry = {"nc": nc, "run": _make_runner(nc)}
        _cache[perm_key] = entry
    return entry


def kernel(x, perm):
    x = np.ascontiguousarray(np.asarray(x), dtype=np.float32)
    perm_np = np.asarray(perm)
    entry = _get_entry(tuple(int(v) for v in perm_np.tolist()))
    return entry["run"](x)


# revision 6
# speedup vs baseline: 68707.3463x; 1.1763x over previous
"""Trainium2 Bass kernel for nn_ChannelRandomPaddingSkip.

Computes out[:, j] = 0.25 * x[:, perm[j]] for x (32, 64, 128, 128) f32,
perm (256,) int32, out (32, 256, 128, 128) f32.

Strategy: pure data-parallel over batch (4 images per core, 8 cores), no
cross-core communication. Per core:
  - SBUF layout: one tile per input channel, [128, 512] f32, partition
    p = (s, b): s in [0,32) segments of the 16384-elem image plane (outer,
    so the DMA engine split sees a large outer dim), b in [0,4) batch.
    Every DMA spans all 128 partitions with 2KiB contiguous runs on both
    the DRAM and SBUF side.
  - 64 channel loads (256KiB each) on the gpsimd (SWDGE) queue, scaled by
    0.25 in place on the vector engine, then 256 gather stores (256KiB
    each) on the sync (HWDGE) queue, source channel baked in host-side
    from the perm values. Stores start as soon as their source channel is
    resident; the Tile scheduler overlaps everything.

Measured (differential wall-clock over an on-device repeat loop, 8 cores
active): ~280us/core vs a ~267us contention roofline (80MiB of HBM traffic
per core at the measured 314GB/s 8-core rate).
"""

import sys

for _p in ("/opt/trn_rl_repo", "/root/.axon_site/_ro/trn_rl_repo"):
    if _p not in sys.path:
        sys.path.append(_p)

import numpy as np

B, C_IN, H, W = 32, 64, 128, 128
C_OUT = 256
N_CORES = 8
B_LOC = B // N_CORES          # 4 batches per core
HW = H * W                    # 16384
SEG = 32                      # segments per image -> 32*4 = 128 partitions
E = HW // SEG                 # 512 elems (2KiB) per segment
H2 = H // SEG                 # rows per segment
SCALE = 0.25

_cache = {}


def _emit_body(nc, mybir, pool, x_v, out_v, by_src):
    for c in range(C_IN):
        if not by_src[c]:
            continue  # channel never gathered; skip the load entirely
        t = pool.tile([128, E], mybir.dt.float32, name=f"ch{c}", tag=f"ch{c}")
        # Loads on SWDGE (gpsimd) keep the HWDGE ring dedicated to stores;
        # measured best with all stores on nc.sync (splitting across the
        # scalar ring was slower).
        nc.gpsimd.dma_start(t[:], x_v[:, :, c, :])
        nc.vector.tensor_scalar_mul(t[:], t[:], SCALE)
        for j in by_src[c]:
            nc.sync.dma_start(out_v[:, :, j, :], t[:])


def build(perm_key, reps=1):
    """Build + compile the per-core program. reps>1 wraps the body in an
    on-device loop (used only by the timing harness)."""
    import concourse.bacc as bacc
    import concourse.tile as tile
    from concourse import mybir

    perm = list(perm_key)
    nc = bacc.Bacc("TRN2", target_bir_lowering=False, debug=False)
    x = nc.dram_tensor("x", [B_LOC, C_IN, H, W], mybir.dt.float32,
                       kind="ExternalInput")
    out = nc.dram_tensor("out", [B_LOC, C_OUT, H, W], mybir.dt.float32,
                         kind="ExternalOutput")

    # (s, b, c, e) views; for a fixed channel the AP is 3-dim
    # DRAM (32, 4, 512) <-> SBUF (128, 512), with 2KiB contiguous runs.
    # s outermost matters: the DMA work split parallelizes the outer dim,
    # and b-outer (size 4) was measured 2.6x slower than s-outer (size 32).
    x_v = x.ap().rearrange("b c (s h2) w -> s b c (h2 w)", s=SEG, h2=H2)
    out_v = out.ap().rearrange("b j (s h2) w -> s b j (h2 w)", s=SEG, h2=H2)

    # Output channels grouped by source channel, so stores can start as soon
    # as their channel is loaded and scaled.
    by_src = [[] for _ in range(C_IN)]
    for j in range(C_OUT):
        by_src[perm[j]].append(j)

    with tile.TileContext(nc) as tc:
        with tc.tile_pool(name="chan", bufs=1) as pool:
            if reps == 1:
                _emit_body(nc, mybir, pool, x_v, out_v, by_src)
            else:
                with tc.For_i(0, reps, 1):
                    _emit_body(nc, mybir, pool, x_v, out_v, by_src)
    nc.compile()
    return nc


class _Entry:
    """Compiled program + cached jit callable for repeat calls."""

    def __init__(self, perm_key):
        import jax
        from concourse import bass2jax
        from concourse.bass_utils import run_bass_kernel_spmd
        from jax.sharding import Mesh, PartitionSpec, NamedSharding

        self.nc = build(perm_key)
        self._jax = jax
        self._first_result = None
        self._sharded = None

        captured = []
        orig_jit = bass2jax.jax.jit

        def spy_jit(*a, **k):
            f = orig_jit(*a, **k)
            captured.append(f)
            return f

        self._capture = (captured, orig_jit, spy_jit, run_bass_kernel_spmd,
                         bass2jax)

        mesh = Mesh(np.asarray(jax.devices()[:N_CORES]), ("core",))
        self._sh = NamedSharding(mesh, PartitionSpec("core"))
        self._zeros_jit = jax.jit(
            lambda: jax.numpy.zeros((B, C_OUT, H, W), np.float32),
            out_shardings=self._sh)

    def run(self, x_full):
        jax = self._jax
        if self._sharded is None:
            # First call: go through run_bass_kernel_spmd (library path) and
            # capture its jit closure for reuse on later calls.
            captured, orig_jit, spy_jit, run_spmd, bass2jax = self._capture
            in_maps = [{"x": x_full[i * B_LOC:(i + 1) * B_LOC]}
                       for i in range(N_CORES)]
            bass2jax.jax.jit = spy_jit
            try:
                res = run_spmd(self.nc, in_maps,
                               core_ids=list(range(N_CORES)))
            finally:
                bass2jax.jax.jit = orig_jit
            self._sharded = captured[-1]
            return np.concatenate(
                [res.results[i]["out"] for i in range(N_CORES)], axis=0)
        zout = self._zeros_jit()          # allocated on device, no transfer
        r = self._sharded(x_full, zout)
        return np.asarray(r[0])


def _get_entry(perm_key):
    entry = _cache.get(perm_key)
    if entry is None:
        entry = _Entry(perm_key)
        _cache[perm_key] = entry
    return entry


def kernel(x, perm):
    x = np.ascontiguousarray(np.asarray(x), dtype=np.float32)
    perm_np = np.asarray(perm)
    entry = _get_entry(tuple(int(v) for v in perm_np.tolist()))
    return entry.run(x)
